# revision 17
# baseline (speedup 1.0000x reference)
"""Trainium2 Bass kernel v3 for nn_DecoderBlock_Mamba.

Sharding: 8 cores = (batch b in 0..3) x (state-half sigma in {0,1}).
Tiled scan layout is INTERLEAVED: partition q = 8*i + j holds channel
d = 16g + i (i = q//8) and state s_lo + j (j = q%8), for channel-group g.
This lets the per-group U/DT replication be ONE DMA (source repeats the
16 group rows 8x via a zero-stride inner dim; partition dim keeps
nonzero stride).

Pipeline: L is processed in NP=2 segments; the front (conv/LN/proj) of
segment s+1 is issued interleaved with the scan phase of segment s, and
the drain/AllReduce of segment s overlaps the scan of s+1. Per-segment
output is post-processed locally (y*silu(z), out_proj, +residual) and
the pairwise AllReduce writes the bf16 output directly.

Self-contained: hardcodes all shapes; no sibling imports.
"""
import numpy as np

C = 64
DI = 128
DS = 16
DR = 4
B = 4
H = 64
W = 64
L = H * W
NG = 8            # channel groups (of 16) per core
NS = 8            # states per core
NCORES = 8
NP = 4            # L segments (pipeline stages)
SEG = L // NP     # 2048
CH = 512
CPS = SEG // CH   # chunks per segment
EPS = 1e-5

# groups whose y-multiply runs on Pool (rest on DVE)
YM_POOL = (2, 6)

_cached = {}


def _build_program(sim=False):
    import concourse.bass as bass
    import concourse.bacc as bacc
    import concourse.mybir as mybir
    import concourse.tile as tile

    dt = mybir.dt
    f32 = dt.float32
    bf16 = dt.bfloat16
    Act = mybir.ActivationFunctionType
    Alu = mybir.AluOpType
    Axis = mybir.AxisListType

    nc = bacc.Bacc(None, target_bir_lowering=False)

    ximgs_d = nc.dram_tensor("ximgs", [C, NP * 5 * SEG], bf16,
                             kind="ExternalInput")
    cf32_d = nc.dram_tensor("cf32", [128, 32], f32, kind="ExternalInput")
    cbf_d = nc.dram_tensor("cbf", [128, 2880], bf16, kind="ExternalInput")
    out_d = nc.dram_tensor("out_b", [4, C, L // 4], bf16,
                       kind="ExternalOutput")

    groups = [[0, 1], [2, 3], [4, 5], [6, 7]]

    with tile.TileContext(nc) as tc:
        with (
            tc.tile_pool(name="dram", bufs=1, space="DRAM") as dpool,
            tc.tile_pool(name="const", bufs=1) as cpool,
            tc.tile_pool(name="big", bufs=1) as bpool,
            tc.tile_pool(name="img", bufs=3) as ipool,
            tc.tile_pool(name="tps", bufs=2) as tpool,
            tc.tile_pool(name="esb", bufs=2) as epool,
            tc.tile_pool(name="sm", bufs=2) as spool,
            tc.tile_pool(name="udt", bufs=2) as udpool,
            tc.tile_pool(name="da", bufs=2) as dapool,
            tc.tile_pool(name="dbx", bufs=2) as xpool,
            tc.tile_pool(name="yp", bufs=5) as ypool,
            tc.tile_pool(name="ys", bufs=2) as yspool,
            tc.tile_pool(name="oc", bufs=2) as ocpool,
            tc.tile_pool(name="ps", bufs=3, space="PSUM") as ps,
            tc.tile_pool(name="ypsA", bufs=4, space="PSUM") as psA,
            tc.tile_pool(name="psy", bufs=1, space="PSUM") as psy,
        ):
            # ---- constants ----
            cf = cpool.tile([128, 32], f32)
            cb = cpool.tile([128, 2880], bf16)
            nc.sync.dma_start(cf[:], cf32_d[:])
            nc.sync.dma_start(cb[:, 0:704], cbf_d[:, 0:704])
            bn_b = cf[0:C, 0:1]
            ip_b1 = cf[:, 1:2]
            cd_b = cf[:, 2:3]
            dt_b = cf[:, 3:4]
            eps_c = cf[:, 4:5]
            mhalf_c = cf[:, 5:6]
            a_vec = cf[:, 8:16]

            ident = cb[:, 0:128]
            cw = cb[0:C, 128:448]
            ip_lhsT = cb[0:C, 448:704]
            M_dt = cb[:, 704:832]
            cdiag = cb[:, 832:1344]
            W_B = cb[:, 1344:1472]
            W_C = cb[:, 1472:1600]
            Rg = cb[:, 1600:2624]
            op_lhsT = cb[:, 2624:2688]
            dpdiag = cb[:, 2688:2816]
            hident = cb[0:C, 2816:2880]

            # ---- persistent activations ----
            SEQ = bpool.tile([C, L], bf16)
            HN = bpool.tile([C, L], bf16)
            XM0 = bpool.tile([DI, L + 4], bf16)   # raw xm, data @ col 4
            ZS = bpool.tile([DI, L], bf16)
            XC = bpool.tile([DI, L], bf16)
            UD = bpool.tile([DI, 2 * L], bf16)    # [U | DT]
            BT = bpool.tile([DI, L], bf16)
            CT = bpool.tile([DI, L], bf16)
            CAR = cpool.tile([128, NP * NG], f32, tag="car")

            o_in = dpool.tile([4, C, L // 4], bf16, tag="oin")
            o_out = dpool.tile([4, C, L // 4], bf16, tag="oout")

            # warm vector clocks
            warm = cpool.tile([128, 1], f32, tag="warm")
            nc.scalar.activation(warm[:], cf[:, 0:1], Act.Copy)
            warm2 = cpool.tile([128, 1], bf16, tag="warm2")
            nc.scalar.activation(warm2[:], cb[:, 0:1], Act.Copy)
            nc.vector.tensor_scalar_mul(XM0[:, 0:4], cf[:, 0:4], 0.0)
            warm3 = cpool.tile([128, 1], bf16, tag="warm3")
            nc.gpsimd.tensor_scalar(warm3[:], cb[:, 0:1], 0.0, None,
                                    op0=Alu.add)

            nc.sync.dma_start(cb[:, 704:2880], cbf_d[:, 704:2880])

            # preload exp/ln table while idle
            dumex = cpool.tile([128, 1], f32, tag="dumex")
            nc.scalar.activation(dumex[:], cf[:, 0:1], Act.Exp)

            UDv = UD[:].rearrange("p (u s c) -> p u s c", u=2, s=NP)

            def front_gen(s):
                # -- conv front + per-chunk LN/backT/HN pipeline --
                chunk_t = []
                for c in range(CPS):
                    im = ipool.tile([C, 5, CH], bf16, tag="img")
                    src = ximgs_d[:, s * 5 * SEG:(s + 1) * 5 * SEG].rearrange(
                        "p (t c) -> p t c", t=5)[:, :, c * CH:(c + 1) * CH]
                    nc.sync.dma_start(im[:], src)
                    gsl = slice(s * SEG + c * CH, s * SEG + (c + 1) * CH)
                    pc = ps.tile([C, CH], f32, tag="mm")
                    for tap in range(5):
                        nc.tensor.matmul(pc[:], cw[:, tap * C:(tap + 1) * C],
                                         im[:, tap, :],
                                         start=(tap == 0), stop=(tap == 4))
                    nc.scalar.activation(SEQ[:, gsl], pc[:], Act.Relu,
                                         bias=bn_b)
                    tps = tpool.tile([128, 4, C], bf16, tag="tps")
                    nc.scalar.dma_start_transpose(tps[:], SEQ[:, gsl])
                    # LN stats on token-major chunk
                    mu = spool.tile([128, 4], f32, tag="mu")
                    nc.vector.tensor_reduce(mu[:], tps[:], Axis.X, Alu.add)
                    sq = tpool.tile([128, 4, C], f32, tag="sq")
                    nc.vector.tensor_tensor(sq[:], tps[:], tps[:],
                                            op=Alu.mult)
                    ss = spool.tile([128, 4], f32, tag="ss")
                    nc.vector.tensor_reduce(ss[:], sq[:], Axis.X, Alu.add)
                    mun = spool.tile([128, 4], f32, tag="mun")
                    nc.vector.tensor_scalar_mul(mun[:], mu[:], 1.0 / C)
                    var = spool.tile([128, 4], f32, tag="var")
                    nc.vector.tensor_scalar(var[:], ss[:], 1.0 / C, EPS,
                                            op0=Alu.mult, op1=Alu.add)
                    mm2 = spool.tile([128, 4], f32, tag="mm2")
                    nc.vector.tensor_tensor(mm2[:], mun[:], mun[:],
                                            op=Alu.mult)
                    nc.vector.tensor_tensor(var[:], var[:], mm2[:],
                                            op=Alu.subtract)
                    rst = spool.tile([128, 4], f32, tag="rst")
                    nc.scalar.activation(rst[:], var[:], Act.Ln, bias=0.0)
                    nc.scalar.activation(rst[:], rst[:], Act.Exp,
                                         scale=mhalf_c)
                    hnt = tpool.tile([128, 4, C], bf16, tag="hnt")
                    nc.vector.tensor_tensor(
                        hnt[:], tps[:],
                        mun[:].unsqueeze(2).to_broadcast((128, 4, C)),
                        op=Alu.subtract)
                    nc.vector.tensor_tensor(
                        hnt[:], hnt[:],
                        rst[:].unsqueeze(2).to_broadcast((128, 4, C)),
                        op=Alu.mult)
                    tb = ps.tile([C, 4, 128], bf16, tag="mm")
                    for k in range(4):
                        nc.tensor.transpose(tb[:, k, :], hnt[:, k, :], ident)
                    nc.vector.tensor_scalar(
                        HN[:, gsl], tb[:].rearrange("p a b -> p (a b)"),
                        0.0, None, op0=Alu.add)
                    chunk_t.append(gsl)
                    if c == 0:
                        yield
                yield
                # -- in_proj (xm copies on ACT; silu block starts) --
                for c in range(CPS):
                    gi = s * SEG + c * CH
                    gsl = slice(gi, gi + CH)
                    xm_ps = ps.tile([DI, CH], f32, tag="mm")
                    nc.tensor.matmul(xm_ps[:], ip_lhsT[0:C, 0:DI],
                                     HN[:, gsl], start=True, stop=True)
                    nc.scalar.activation(XM0[:, 4 + gi:4 + gi + CH], xm_ps[:],
                                         Act.Copy)
                    z_ps = ps.tile([DI, CH], f32, tag="mm")
                    nc.tensor.matmul(z_ps[:], ip_lhsT[0:C, DI:2 * DI],
                                     HN[:, gsl], start=True, stop=True)
                    nc.scalar.activation(ZS[:, gsl], z_ps[:], Act.Silu,
                                         bias=ip_b1)
                yield
                # -- conv1d + silu --
                for c in range(CPS):
                    gi = s * SEG + c * CH
                    gsl = slice(gi, gi + CH)
                    cc = ps.tile([DI, CH], f32, tag="mm")
                    for tap in range(4):
                        nc.tensor.matmul(cc[:],
                                         cdiag[:, tap * 128:(tap + 1) * 128],
                                         XM0[:, 1 + tap + gi:1 + tap + gi + CH],
                                         start=(tap == 0), stop=(tap == 3))
                    nc.scalar.activation(XC[:, gsl], cc[:], Act.Silu,
                                         bias=cd_b)
                yield
                # -- B/C tiles (BT first: gates dbx) --
                for c in range(CPS):
                    gsl = slice(s * SEG + c * CH, s * SEG + (c + 1) * CH)
                    bt_ps = ps.tile([DI, CH], f32, tag="mm")
                    nc.tensor.matmul(bt_ps[:], W_B, XC[:, gsl],
                                     start=True, stop=True)
                    nc.vector.tensor_scalar(BT[:, gsl], bt_ps[:], 0.0, None,
                                            op0=Alu.add)
                    ct_ps = ps.tile([DI, CH], f32, tag="mm")
                    nc.tensor.matmul(ct_ps[:], W_C, XC[:, gsl],
                                     start=True, stop=True)
                    nc.vector.tensor_scalar(CT[:, gsl], ct_ps[:], 0.0, None,
                                            op0=Alu.add)
                yield
                # -- x_proj dt + esb exp + DT + U --
                ESB = epool.tile([DI, SEG], bf16, tag="esb")
                for c in range(CPS):
                    gsl = slice(s * SEG + c * CH, s * SEG + (c + 1) * CH)
                    dt_ps = ps.tile([DI, CH], f32, tag="mm")
                    nc.tensor.matmul(dt_ps[:], M_dt, XC[:, gsl],
                                     start=True, stop=True)
                    nc.scalar.activation(ESB[:, c * CH:(c + 1) * CH],
                                         dt_ps[:], Act.Exp, bias=dt_b)
                nc.scalar.activation(UDv[:, 1, s, :], ESB[:], Act.Ln,
                                     bias=1.0)
                nc.vector.tensor_mul(UDv[:, 0, s, :], UDv[:, 1, s, :],
                                     XC[:, s * SEG:(s + 1) * SEG])
                yield

            def repl_dmas(s):
                UT = udpool.tile([128, NG, 2, SEG], bf16, tag="udt")
                for j in range(8):
                    dst = UT[16 * j:16 * (j + 1), :, :, :]
                    for u in range(2):
                        sap = UDv[:, u, s, :].rearrange(
                            "(g i) c -> i g c", g=NG)
                        eng = nc.sync if j < 4 else nc.scalar
                        eng.dma_start(dst[:, :, u, :], sap)
                return UT

            def scan_bundle(s, g, UT, yps):
                ssl = slice(s * SEG, (s + 1) * SEG)
                dbx = xpool.tile([DI, SEG], bf16, tag="dbx")
                nc.vector.tensor_tensor(dbx[:], UT[:, g, 0, :], BT[:, ssl],
                                        op=Alu.mult)
                dA = dapool.tile([DI, SEG], f32, tag="dA")
                nc.scalar.activation(dA[:], UT[:, g, 1, :], Act.Exp,
                                     scale=a_vec[:, g:g + 1])
                yp = ypool.tile([DI, SEG], bf16, tag="yp")
                init = 0.0 if s == 0 else CAR[:, (s - 1) * NG + g:
                                             (s - 1) * NG + g + 1]
                nc.vector.tensor_tensor_scan(yp[:], dA[:], dbx[:], init,
                                             op0=Alu.mult, op1=Alu.add)
                if s < NP - 1:
                    nc.vector.tensor_scalar(
                        CAR[:, s * NG + g:s * NG + g + 1],
                        yp[:, SEG - 1:SEG], 0.0, None, op0=Alu.add)
                if not (s == NP - 1 and g >= 4):
                    nc.gpsimd.tensor_tensor(yp[:], yp[:], CT[:, ssl],
                                            op=Alu.mult)
                else:
                    nc.vector.tensor_tensor(yp[:], yp[:], CT[:, ssl],
                                            op=Alu.mult)
                # y reduce: 4 chunk matmuls into this segment's PSUM accum
                for ci in range(CPS):
                    nc.tensor.matmul(yps[ci][:],
                                     Rg[:, g * 128:(g + 1) * 128],
                                     yp[:, ci * CH:(ci + 1) * CH],
                                     start=(g == 0), stop=False)
                return yp

            def dp_fold(s, yps):
                for ci in range(CPS):
                    gsl = slice(s * SEG + ci * CH, s * SEG + (ci + 1) * CH)
                    nc.tensor.matmul(yps[ci][:], dpdiag, XC[:, gsl],
                                     start=False, stop=True)

            def drain_gen(s, yps):
                LQ = L // 4
                for ci in range(CPS):
                    gi = s * SEG + ci * CH
                    gsl = slice(gi, gi + CH)
                    qg = gi // LQ
                    qo = gi % LQ
                    ysb = yspool.tile([DI, CH], bf16, tag="ysb")
                    nc.vector.tensor_tensor(ysb[:], yps[ci][:],
                                            ZS[:, gsl], op=Alu.mult)
                    op_ps = psy.tile([C, CH], f32, tag="op")
                    nc.tensor.matmul(op_ps[:], op_lhsT, ysb[:],
                                     start=True, stop=False)
                    nc.tensor.matmul(op_ps[:], hident, SEQ[:, gsl],
                                     start=False, stop=True)
                    outc = ocpool.tile([C, CH], bf16, tag="outc")
                    nc.vector.tensor_scalar(outc[:], op_ps[:], 0.0, None,
                                            op0=Alu.add)
                    nc.scalar.dma_start(
                        o_in[qg, :, qo:qo + CH], outc[:])
                    if (gi + CH) % LQ == 0:
                        if sim:
                            nc.sync.dma_start(o_out[qg], o_in[qg])
                        else:
                            nc.gpsimd.collective_compute(
                                "AllReduce", Alu.add, replica_groups=groups,
                                ins=[o_in[qg].opt()],
                                outs=[o_out[qg].opt()])
                        nc.sync.dma_start(out_d[qg], o_out[qg])
                    yield

            # ================= main schedule =================
            fg = front_gen(0)
            for _ in fg:
                pass
            udts = repl_dmas(0)
            yps = [psA.tile([DI, CH], f32, name=f"yps0_{i}", tag="ypsA")
                   for i in range(CPS)]
            prev_drain = None
            for seg in range(NP):
                fg_next = front_gen(seg + 1) if seg + 1 < NP else None
                for g in range(NG):
                    scan_bundle(seg, g, udts, yps)
                    if fg_next is not None:
                        next(fg_next, None)
                    if prev_drain is not None:
                        next(prev_drain, None)
                if fg_next is not None:
                    for _ in fg_next:
                        pass
                dp_fold(seg, yps)
                if prev_drain is not None:
                    for _ in prev_drain:
                        pass
                if seg + 1 < NP:
                    udts = repl_dmas(seg + 1)
                    new_yps = [psA.tile([DI, CH], f32,
                                        name=f"yps{seg + 1}_{i}", tag="ypsA")
                               for i in range(CPS)]
                prev_drain = drain_gen(seg, yps)
                if seg + 1 < NP:
                    yps = new_yps
                else:
                    for _ in prev_drain:
                        pass

    nc.compile()
    return nc


def _host_precompute(inp):
    import ml_dtypes
    f = lambda k: np.asarray(inp[k], np.float32)
    bf = lambda a: np.ascontiguousarray(a.astype(ml_dtypes.bfloat16))
    w1 = f("conv_w")[:, :, 0, 0]
    wh = f("dwh_w")[:, 0, :, 0]
    ww = f("dww_w")[:, 0, 0, :]
    s_bn = f("bn_g") / np.sqrt(f("bn_v") + EPS)
    taps = [
        w1 * (1.0 + wh[:, 1] + ww[:, 1])[None, :],
        w1 * wh[:, 0][None, :],
        w1 * wh[:, 2][None, :],
        w1 * ww[:, 0][None, :],
        w1 * ww[:, 2][None, :],
    ]
    cw = np.concatenate([t.T for t in taps], axis=1)
    cw = cw * np.tile(s_bn, 5)[None, :]
    btot = f("conv_b") + w1 @ (f("dwh_b") + f("dww_b"))
    bn_bias = s_bn * (btot - f("bn_m")) + f("bn_b")
    ipw = f("in_proj_w")
    ip_lhsT = (ipw * f("ln_g")[None, :]).T            # [64, 256]
    ip_bias = ipw @ f("ln_b")                          # [256]
    xpw = f("x_proj_w")                                # [36, 128]
    M_dt = f("dt_proj_w") @ xpw[:DR]                   # [128, 128]
    a_full = -np.exp(np.asarray(inp["A_log"], np.float32))
    cdw = f("convd_w")[:, 0, :]                        # [128, 4]
    # conv1d bias with in_proj xm-bias folded through the taps
    cd_bias = f("convd_b") + ip_bias[:DI] * cdw.sum(axis=1)

    per_sigma = []
    q = np.arange(128)
    for sg in range(2):
        s_lo = sg * NS
        cf32 = np.zeros((128, 32), np.float32)
        cf32[:C, 0] = bn_bias
        cf32[:, 1] = ip_bias[DI:]
        cf32[:, 2] = cd_bias
        cf32[:, 3] = f("dt_proj_b")
        cf32[:, 4] = EPS
        cf32[:, 5] = -0.5
        for g in range(NG):
            cf32[:, 8 + g] = a_full[16 * g + q % 16, s_lo + q // 16]

        cbf = np.zeros((128, 2880), np.float32)
        cbf[:, 0:128] = np.eye(128, dtype=np.float32)
        cbf[:C, 128:448] = cw
        cbf[:C, 448:704] = ip_lhsT
        cbf[:, 704:832] = M_dt.T
        for tap in range(4):
            cbf[:, 832 + tap * 128:832 + (tap + 1) * 128] = \
                np.diag(cdw[:, tap])
        for qq in range(128):
            cbf[:, 1344 + qq] = xpw[DR + s_lo + qq // 16]
            cbf[:, 1472 + qq] = xpw[DR + DS + s_lo + qq // 16]
        for g in range(NG):
            for qq in range(128):
                cbf[qq, 1600 + g * 128 + 16 * g + qq % 16] = 1.0
        cbf[:, 2624:2688] = f("out_proj_w").T
        cbf[:, 2688:2816] = np.diag(0.5 * f("Dp"))
        cbf[:C, 2816:2880] = 0.5 * np.eye(C, dtype=np.float32)
        per_sigma.append(dict(cf32=cf32, cbf=bf(cbf)))
    return per_sigma


def _shift_images(xb):
    # 5 pre-shifted copies: ctr, up(reads h-1), dn(h+1), lf(w-1), rt(w+1)
    import ml_dtypes
    out = np.zeros((5, C, H, W), np.float32)
    out[0] = xb
    out[1, :, 1:, :] = xb[:, :-1, :]
    out[2, :, :-1, :] = xb[:, 1:, :]
    out[3, :, :, 1:] = xb[:, :, :-1]
    out[4, :, :, :-1] = xb[:, :, 1:]
    imgs = out.reshape(5, C, L)
    # pack per segment: [C, NP, 5, SEG]
    arr = np.empty((C, NP, 5, SEG), np.float32)
    for s in range(NP):
        arr[:, s] = imgs[:, :, s * SEG:(s + 1) * SEG].transpose(1, 0, 2)
    return np.ascontiguousarray(
        arr.reshape(C, NP * 5 * SEG).astype(ml_dtypes.bfloat16))


TRACE = False
LAST_EXEC_NS = None
LAST_TRACE_DIR = None


def kernel(**inputs):
    global LAST_EXEC_NS, LAST_TRACE_DIR
    from concourse.bass_utils import run_bass_kernel_spmd

    if "nc" not in _cached:
        _cached["nc"] = _build_program()
    nc = _cached["nc"]

    per_sigma = _host_precompute(inputs)
    x = np.asarray(inputs["x"], np.float32)
    in_maps = []
    for c in range(NCORES):
        b, sg = c // 2, c % 2
        m = dict(per_sigma[sg])
        m["ximgs"] = _shift_images(x[b])
        in_maps.append(m)

    kw = {}
    if TRACE:
        import tempfile
        LAST_TRACE_DIR = tempfile.mkdtemp(prefix="bass_trace_")
        kw = dict(trace=True, tmpdir=LAST_TRACE_DIR)
    r = run_bass_kernel_spmd(nc, in_maps, list(range(NCORES)), **kw)
    if r.exec_time_ns is not None:
        LAST_EXEC_NS = r.exec_time_ns
    res = r.results
    out = np.empty((B, C, H, W), np.float32)
    for b in range(B):
        ob = np.asarray(res[2 * b]["out_b"]).astype(np.float32)
        out[b] = ob.transpose(1, 0, 2).reshape(C, H, W)
    return out
